# revision 23
# baseline (speedup 1.0000x reference)
"""Baichuan attention on 8 Trainium2 NeuronCores — tensor-parallel over heads.

Sharding: core c computes heads [4c, 4c+4): its slice of the fused QKV
projection, attention for those heads, then 1/8 of o_proj's output columns
after an AllGather of the per-core context slices (moves 4MB/rank instead of
a 32MB AllReduce of partial sums; mathematically identical to the module's
world_size logic).

Layout: scores are computed transposed (scoresT[k, q] blocks) so the PE
contraction dim always sits on SBUF partitions and every matmul streams a
512-wide moving operand. Matmul operands are fp16 (1 cyc/row on the PE) with
fp32 PSUM accumulation — measured end-to-end error vs the fp32 reference is
~6e-4 absmax-relative, on par with the f32r (tf32) path. The AllGather is
chunked over four s_q blocks so collective latency and o_proj overlap the
attention of later blocks, keeping the PE stream dense (HAM stays warm).
"""

import numpy as np

import concourse.bacc as bacc
import concourse.mybir as mybir
import concourse.tile as tile
from concourse.bass_utils import run_bass_kernel_spmd

F32 = mybir.dt.float32

N_CORES = 8
NUM_HEADS = 32
HEAD_DIM = 128
P = 128          # SBUF partitions / PE contraction tile
SQ = 512         # s_q block width (PSUM bank = 512 fp32)
MM_MODE = "f16"  # 'f16' | 'f32' (operand dtype for matmuls)

_CACHE: dict = {}


def _mm_dtype(mode):
    return {"f16": mybir.dt.float16, "f32": F32}[mode]


def build(S, H, block_cls, mode=MM_MODE):
    """Build the SPMD program. block_cls[(t, b)] = 'plain' | 'mask' for every
    computed scoresT block ([128 s_k] x [SQ s_q]); absent = fully masked, skip.
    """
    MD = _mm_dtype(mode)
    hpc = NUM_HEADS // N_CORES          # heads per core
    dpc = hpc * HEAD_DIM                # per-core slice of the hidden dim
    n_ht = H // P                       # contraction tiles for QKV/o_proj
    n_qk = 2 * dpc // P                 # q+k output tiles
    n_sq = S // SQ                      # s_q blocks
    n_st = S // P                       # s_k tiles
    scale = 1.0 / np.sqrt(np.float32(HEAD_DIM))
    s_half = S // 2
    sb_per_half = s_half // SQ

    nc = bacc.Bacc("TRN2", target_bir_lowering=False, debug=False,
                   num_devices=N_CORES)

    xT = nc.dram_tensor("xT", [H, S], MD, kind="ExternalInput")
    wqkT = nc.dram_tensor("wqkT", [H, 2 * dpc], MD, kind="ExternalInput")
    wvT = nc.dram_tensor("wvT", [H, dpc], MD, kind="ExternalInput")
    maskT = nc.dram_tensor("maskT", [S, S], F32, kind="ExternalInput")
    woT = nc.dram_tensor("woT", [H, dpc], MD, kind="ExternalInput")
    out_cols = nc.dram_tensor("out_cols", [S, dpc], F32, kind="ExternalOutput")

    # AllGather in head-pair chunks: gat[b][pp] holds local heads
    # {2pp, 2pp+1} for s_q block b; ct[b][pp] gathers those pairs from all
    # ranks. o_proj consumes them against host-permuted w_o rows.
    gat_b = [[nc.dram_tensor(f"gat_{b}_{pp}", [dpc // 2, SQ], MD)
              for pp in range(2)] for b in range(n_sq)]
    ct_b = [[nc.dram_tensor(f"ct_{b}_{pp}", [H // 2, SQ], MD,
                            addr_space="Shared") for pp in range(2)]
            for b in range(n_sq)]

    xT_t = xT.ap().rearrange("(t p) s -> p t s", p=P)
    wqkT_t = wqkT.ap().rearrange("(t p) o -> p t o", p=P)
    wvT_t = wvT.ap().rearrange("(t p) o -> p t o", p=P)
    woT_t = woT.ap().rearrange("(t p) j -> p t j", p=P)

    # sorted by (b, t) so block b=0's diagonal tiles arrive first
    mask_blocks = sorted({k for k, v in block_cls.items() if v == "mask"},
                         key=lambda k: (k[1], k[0]))
    mask_slot = {k: i for i, k in enumerate(mask_blocks)}

    with tile.TileContext(nc) as tc:
        with (
            tc.tile_pool(name="consts", bufs=1) as cpool,
            tc.tile_pool(name="span", bufs=1) as span,
        ):
            ones_f = cpool.tile([P, P], F32, tag="ones_f")
            nc.gpsimd.memset(ones_f[:], 1.0)
            ones_sq = cpool.tile([P, P], MD, tag="ones_sq")
            nc.scalar.copy(ones_sq[:], ones_f[:])

            # v ([s_k, d] natural, all heads) and q/k (transposed, all heads)
            # live in SBUF across phases 1-2; QKV evictions write them
            # directly (no DRAM bounce)
            v_sb = span.tile([P, n_st, dpc], MD, tag="v")
            qk_all = span.tile([P, n_qk, S], MD, tag="qk")

            # =============== phase 1: QKV projection ===============
            # q/k in transposed orientation -> DRAM scratch; v in natural
            # orientation (x stationary, Wv moving) -> resident v_sb.
            with (
                tc.tile_pool(name="qkv_x", bufs=1) as xpool,
                tc.tile_pool(name="qkv_w", bufs=3) as wpool,
                tc.tile_pool(name="qkv_wv", bufs=1) as wvpool,
                tc.tile_pool(name="qkv_ps", bufs=4, space="PSUM") as pspool,
            ):
                wv_sb = wvpool.tile([P, n_ht, dpc], MD, tag="wv")
                for half in range(2):
                    x_tile = xpool.tile([P, n_ht, s_half], MD, tag="x")
                    for t in range(n_ht):
                        nc.sync.dma_start(
                            x_tile[:, t, :],
                            xT_t[:, t, half * s_half:(half + 1) * s_half])
                    # q/k first (their W tiles are small, so the PE starts
                    # within a few us); wv's 4MB DMA is emitted after the
                    # first w tile so it doesn't gate startup
                    for ot in range(n_qk):
                        w_tile = wpool.tile([P, n_ht, P], MD, tag="w")
                        nc.sync.dma_start(
                            w_tile[:], wqkT_t[:, :, ot * P:(ot + 1) * P])
                        if half == 0 and ot == 1:
                            nc.sync.dma_start(wv_sb[:], wvT_t[:])
                        for sb in range(sb_per_half):
                            ps = pspool.tile([P, SQ], F32, tag="qkv")
                            for t in range(n_ht):
                                nc.tensor.matmul(
                                    ps[:],
                                    w_tile[:, t, :],
                                    x_tile[:, t, sb * SQ:(sb + 1) * SQ],
                                    start=(t == 0), stop=(t == n_ht - 1))
                            # fold the softmax scale into q at eviction;
                            # write straight into the resident qk tile
                            mul = scale if ot < dpc // P else 1.0
                            lo = half * s_half + sb * SQ
                            nc.scalar.mul(qk_all[:, ot, lo:lo + SQ],
                                          ps[:], mul)
                    # v: psum [s=128, dpc] accumulated over h-tiles
                    for sti in range(s_half // P):
                        st_g = half * (s_half // P) + sti
                        ps_v = pspool.tile([P, dpc], F32, tag="qkv")
                        for t in range(n_ht):
                            nc.tensor.matmul(
                                ps_v[:],
                                x_tile[:, t, sti * P:(sti + 1) * P],
                                wv_sb[:, t, :],
                                start=(t == 0), stop=(t == n_ht - 1))
                        nc.vector.tensor_copy(v_sb[:, st_g, :], ps_v[:])

            # ====== phases 2-4: attention / chunked AllGather / o_proj ======
            with (
                tc.tile_pool(name="at_mask", bufs=1) as mpool,
                tc.tile_pool(name="at_exp", bufs=3) as epool,
                tc.tile_pool(name="at_out", bufs=3) as opool,
                tc.tile_pool(name="at_r", bufs=2) as rpool,
                tc.tile_pool(name="op_w", bufs=1) as owpool,
                tc.tile_pool(name="op_ct", bufs=40) as ctpool,
                tc.tile_pool(name="op_stage", bufs=4) as ospool,
                tc.tile_pool(name="at_ps", bufs=2, space="PSUM") as aps,
            ):
                # attention-output and o_proj psum share one 4-slot rotation
                # (scores 2 + ao 4 + row 2 = 8 banks); the deep ao rotation
                # hides the reciprocal latency on the normalize chain
                def ao_tile():
                    return aps.tile([P, SQ], F32, tag="ao", bufs=4,
                                    name="ao_ps")
                if mask_blocks:
                    mtile = mpool.tile([P, len(mask_blocks), SQ], F32,
                                       tag="mask")
                    for (t, b), i in mask_slot.items():
                        nc.sync.dma_start(
                            mtile[:, i, :],
                            maskT.ap()[t * P:(t + 1) * P,
                                       b * SQ:(b + 1) * SQ])
                wo_sb = owpool.tile([P, n_ht, dpc], MD, tag="wo")
                nc.sync.dma_start(wo_sb[:], woT_t[:])

                for b in range(n_sq):
                    ts_here = [t for t in range(n_st) if (t, b) in block_cls]
                    for h in range(hpc):
                        q_sl = qk_all[:, h, b * SQ:(b + 1) * SQ]
                        ps_o = ao_tile()
                        ps_row = aps.tile([P, SQ], F32, tag="row")
                        for i, t in enumerate(ts_here):
                            ps_s = aps.tile([P, SQ], F32, tag="scores")
                            nc.tensor.matmul(
                                ps_s[:],
                                qk_all[:, hpc + h, t * P:(t + 1) * P],
                                q_sl, start=True, stop=True)
                            if block_cls[(t, b)] == "mask":
                                nc.vector.tensor_add(
                                    ps_s[:], ps_s[:],
                                    mtile[:, mask_slot[(t, b)], :])
                            ex = epool.tile([P, SQ], MD, tag="exp")
                            nc.scalar.activation(
                                ex[:], ps_s[:],
                                mybir.ActivationFunctionType.Exp)
                            first, last = i == 0, i == len(ts_here) - 1
                            nc.tensor.matmul(
                                ps_o[:], v_sb[:, t, h * P:(h + 1) * P],
                                ex[:], start=first, stop=last)
                            # rowsum broadcast to all partitions via the
                            # all-ones stationary operand
                            nc.tensor.matmul(
                                ps_row[:], ones_sq[:], ex[:],
                                start=first, stop=last)
                        recip = rpool.tile([P, SQ], F32, tag="recip")
                        nc.vector.reciprocal(recip[:], ps_row[:])
                        ob = opool.tile([P, SQ], MD, tag="ob")
                        nc.vector.tensor_mul(ob[:], ps_o[:], recip[:])
                        nc.sync.dma_start(
                            gat_b[b][h // 2].ap()[(h % 2) * P:
                                                  (h % 2 + 1) * P, :], ob[:])
                        if h % 2 == 1:
                            nc.gpsimd.collective_compute(
                                "AllGather", mybir.AluOpType.bypass,
                                replica_groups=[list(range(N_CORES))],
                                ins=[gat_b[b][h // 2].ap().opt()],
                                outs=[ct_b[b][h // 2].ap().opt()])

                    # o_proj for this s_q block (overlaps later blocks);
                    # k-tile t of the half-gathers pairs with the
                    # host-permuted w_o row block t
                    cts = []
                    for pp in range(2):
                        ct_t = ct_b[b][pp].ap().rearrange(
                            "(t p) s -> p t s", p=P)
                        for t in range(n_ht // 2):
                            c_t = ctpool.tile([P, SQ], MD, tag="ct")
                            nc.sync.dma_start(c_t[:], ct_t[:, t, :])
                            cts.append(c_t)
                    for st in range(SQ // P):
                        ps = ao_tile()
                        for t in range(n_ht):
                            nc.tensor.matmul(
                                ps[:],
                                cts[t][:, st * P:(st + 1) * P],
                                wo_sb[:, t, :],
                                start=(t == 0), stop=(t == n_ht - 1))
                        ob = ospool.tile([P, dpc], F32, tag="ostage")
                        nc.scalar.copy(ob[:], ps[:])
                        nc.sync.dma_start(
                            out_cols.ap()[b * SQ + st * P:
                                          b * SQ + (st + 1) * P, :], ob[:])

    nc.compile()
    return nc


def _classify_blocks(maskT_np, S):
    """Classify each [128, SQ] scoresT block of the (transposed) mask."""
    cls = {}
    for t in range(S // P):
        rows = maskT_np[t * P:(t + 1) * P]
        for b in range(S // SQ):
            blk = rows[:, b * SQ:(b + 1) * SQ]
            if np.all(blk <= -1e30):
                continue                      # fully masked: skip compute
            if np.all(blk == 0.0):
                cls[(t, b)] = "plain"
            else:
                cls[(t, b)] = "mask"
    return cls


def make_in_maps(hidden_states, attention_mask, w_pack, w_o):
    B, S, H = hidden_states.shape
    hpc = NUM_HEADS // N_CORES
    dpc = hpc * HEAD_DIM
    np_md = mybir.dt.np(_mm_dtype(MM_MODE))
    xT = np.ascontiguousarray(hidden_states[0].T).astype(np_md)
    maskT_np = np.ascontiguousarray(
        np.broadcast_to(attention_mask, (1, 1, S, S))[0, 0].T,
        dtype=np.float32)
    # w_o rows permuted to match the head-pair AllGather layout:
    # [pp][rank][head-in-pair] blocks of 128
    perm = np.concatenate(
        [np.arange(128 * (4 * r + 2 * pp + hh),
                   128 * (4 * r + 2 * pp + hh) + 128)
         for pp in (0, 1) for r in range(N_CORES) for hh in (0, 1)])
    in_maps = []
    for c in range(N_CORES):
        sl = slice(c * dpc, (c + 1) * dpc)
        wqk_c = np.concatenate(
            [w_pack[0 * H:1 * H][sl], w_pack[1 * H:2 * H][sl]], axis=0)
        woT_c = np.ascontiguousarray(w_o[sl].T)[perm]
        in_maps.append({
            "xT": xT,
            "wqkT": np.ascontiguousarray(wqk_c.T).astype(np_md),
            "wvT": np.ascontiguousarray(w_pack[2 * H:3 * H][sl].T
                                        ).astype(np_md),
            "maskT": maskT_np,
            "woT": np.ascontiguousarray(woT_c).astype(np_md),
        })
    return in_maps, maskT_np


def kernel(hidden_states, attention_mask, w_pack, w_o):
    B, S, H = hidden_states.shape
    assert B == 1 and H == NUM_HEADS * HEAD_DIM
    assert S % (2 * SQ) == 0

    in_maps, maskT_np = make_in_maps(hidden_states, attention_mask,
                                     w_pack, w_o)
    block_cls = _classify_blocks(maskT_np, S)

    key = (S, H, tuple(sorted(block_cls.items())), MM_MODE)
    if key not in _CACHE:
        _CACHE[key] = build(S, H, block_cls, MM_MODE)
    nc = _CACHE[key]

    res = run_bass_kernel_spmd(nc, in_maps, core_ids=list(range(N_CORES)))
    out = np.concatenate(
        [res.results[c]["out_cols"] for c in range(N_CORES)], axis=1)
    return out.reshape(1, S, H).astype(np.float32)


# revision 30
# speedup vs baseline: 1.1081x; 1.1081x over previous
"""Baichuan attention on 8 Trainium2 NeuronCores — tensor-parallel over heads.

Sharding: core c computes heads [4c, 4c+4): its slice of the fused QKV
projection, attention for those heads, then 1/8 of o_proj's output columns
after an AllGather of the per-core context slices (moves 4MB/rank instead of
a 32MB AllReduce of partial sums; mathematically identical to the module's
world_size logic).

Layout: scores are computed transposed (scoresT[k, q] blocks) so the PE
contraction dim always sits on SBUF partitions and every matmul streams a
512-wide moving operand. Matmul operands are fp16 (1 cyc/row on the PE) with
fp32 PSUM accumulation — measured end-to-end error vs the fp32 reference is
~6e-4 absmax-relative, on par with the f32r (tf32) path. The AllGather is
chunked over four s_q blocks so collective latency and o_proj overlap the
attention of later blocks, keeping the PE stream dense (HAM stays warm).
"""

import numpy as np

import concourse.bacc as bacc
import concourse.mybir as mybir
import concourse.tile as tile
from concourse.bass_utils import run_bass_kernel_spmd

F32 = mybir.dt.float32

N_CORES = 8
NUM_HEADS = 32
HEAD_DIM = 128
P = 128          # SBUF partitions / PE contraction tile
SQ = 512         # s_q block width (PSUM bank = 512 fp32)
MM_MODE = "f16"  # 'f16' | 'f32' (operand dtype for matmuls)

_CACHE: dict = {}


def _mm_dtype(mode):
    return {"f16": mybir.dt.float16, "f32": F32}[mode]


def build(S, H, block_cls, mode=MM_MODE):
    """Build the SPMD program. block_cls[(t, b)] = 'plain' | 'mask' for every
    computed scoresT block ([128 s_k] x [SQ s_q]); absent = fully masked, skip.
    """
    MD = _mm_dtype(mode)
    hpc = NUM_HEADS // N_CORES          # heads per core
    dpc = hpc * HEAD_DIM                # per-core slice of the hidden dim
    n_ht = H // P                       # contraction tiles for QKV/o_proj
    n_qk = 2 * dpc // P                 # q+k output tiles
    n_sq = S // SQ                      # s_q blocks
    n_st = S // P                       # s_k tiles
    scale = 1.0 / np.sqrt(np.float32(HEAD_DIM))
    s_half = S // 2
    sb_per_half = s_half // SQ

    nc = bacc.Bacc("TRN2", target_bir_lowering=False, debug=False,
                   num_devices=N_CORES)

    xT = nc.dram_tensor("xT", [H, S], MD, kind="ExternalInput")
    wqkT = nc.dram_tensor("wqkT", [H, 2 * dpc], MD, kind="ExternalInput")
    wvT = nc.dram_tensor("wvT", [H, dpc], MD, kind="ExternalInput")
    maskT = nc.dram_tensor("maskT", [S, S], F32, kind="ExternalInput")
    woT = nc.dram_tensor("woT", [H, dpc], MD, kind="ExternalInput")
    out_cols = nc.dram_tensor("out_cols", [S, dpc], F32, kind="ExternalOutput")

    # AllGather in head-pair chunks: gat[b][pp] holds local heads
    # {2pp, 2pp+1} for s_q block b; ct[b][pp] gathers those pairs from all
    # ranks. o_proj consumes them against host-permuted w_o rows.
    gat_b = [[nc.dram_tensor(f"gat_{b}_{pp}", [dpc // 2, SQ], MD)
              for pp in range(2)] for b in range(n_sq)]
    ct_b = [[nc.dram_tensor(f"ct_{b}_{pp}", [H // 2, SQ], MD,
                            addr_space="Shared") for pp in range(2)]
            for b in range(n_sq)]

    xT_t = xT.ap().rearrange("(t p) s -> p t s", p=P)
    wqkT_t = wqkT.ap().rearrange("(t p) o -> p t o", p=P)
    wvT_t = wvT.ap().rearrange("(t p) o -> p t o", p=P)
    woT_t = woT.ap().rearrange("(t p) j -> p t j", p=P)

    # sorted by (b, t) so block b=0's diagonal tiles arrive first
    mask_blocks = sorted({k for k, v in block_cls.items() if v == "mask"},
                         key=lambda k: (k[1], k[0]))
    mask_slot = {k: i for i, k in enumerate(mask_blocks)}

    with tile.TileContext(nc) as tc:
        with (
            tc.tile_pool(name="consts", bufs=1) as cpool,
            tc.tile_pool(name="span", bufs=1) as span,
        ):
            ones_f = cpool.tile([P, P], F32, tag="ones_f")
            nc.gpsimd.memset(ones_f[:], 1.0)
            ones_sq = cpool.tile([P, P], MD, tag="ones_sq")
            nc.scalar.copy(ones_sq[:], ones_f[:])

            # v ([s_k, d] natural, all heads) and q/k (transposed, all heads)
            # live in SBUF across phases 1-2; QKV evictions write them
            # directly (no DRAM bounce)
            v_sb = span.tile([P, n_st, dpc], MD, tag="v")
            qk_all = span.tile([P, n_qk, S], MD, tag="qk")

            # =============== phase 1: QKV projection ===============
            # q/k in transposed orientation -> DRAM scratch; v in natural
            # orientation (x stationary, Wv moving) -> resident v_sb.
            with (
                tc.tile_pool(name="qkv_x", bufs=1) as xpool,
                tc.tile_pool(name="qkv_w", bufs=3) as wpool,
                tc.tile_pool(name="qkv_wv", bufs=1) as wvpool,
                tc.tile_pool(name="qkv_ps", bufs=4, space="PSUM") as pspool,
            ):
                wv_sb = wvpool.tile([P, n_ht, dpc], MD, tag="wv")
                for half in range(2):
                    # first q/k weight tile ahead of the bulk x transfer so
                    # the PE starts within a few us; wv's 4MB comes after the
                    # second w tile
                    w_tiles = {}
                    w_tiles[0] = wpool.tile([P, n_ht, P], MD, tag="w",
                                            name="w_tile")
                    nc.sync.dma_start(
                        w_tiles[0][:], wqkT_t[:, :, 0 * P:1 * P])
                    x_tile = xpool.tile([P, n_ht, s_half], MD, tag="x")
                    for t in range(n_ht):
                        nc.sync.dma_start(
                            x_tile[:, t, :],
                            xT_t[:, t, half * s_half:(half + 1) * s_half])
                    for ot in range(n_qk):
                        if ot not in w_tiles:
                            w_tiles[ot] = wpool.tile([P, n_ht, P], MD,
                                                     tag="w", name="w_tile")
                            nc.sync.dma_start(
                                w_tiles[ot][:],
                                wqkT_t[:, :, ot * P:(ot + 1) * P])
                        w_tile = w_tiles[ot]
                        if half == 0 and ot == 1:
                            nc.sync.dma_start(wv_sb[:], wvT_t[:])
                        for sb in range(sb_per_half):
                            ps = pspool.tile([P, SQ], F32, tag="qkv")
                            for t in range(n_ht):
                                nc.tensor.matmul(
                                    ps[:],
                                    w_tile[:, t, :],
                                    x_tile[:, t, sb * SQ:(sb + 1) * SQ],
                                    start=(t == 0), stop=(t == n_ht - 1))
                            # fold the softmax scale into q at eviction;
                            # write straight into the resident qk tile
                            mul = scale if ot < dpc // P else 1.0
                            lo = half * s_half + sb * SQ
                            nc.scalar.mul(qk_all[:, ot, lo:lo + SQ],
                                          ps[:], mul)
                    # v: psum [s=128, dpc] accumulated over h-tiles
                    for sti in range(s_half // P):
                        st_g = half * (s_half // P) + sti
                        ps_v = pspool.tile([P, dpc], F32, tag="qkv")
                        for t in range(n_ht):
                            nc.tensor.matmul(
                                ps_v[:],
                                x_tile[:, t, sti * P:(sti + 1) * P],
                                wv_sb[:, t, :],
                                start=(t == 0), stop=(t == n_ht - 1))
                        nc.vector.tensor_copy(v_sb[:, st_g, :], ps_v[:])

            # ====== phases 2-4: attention / chunked AllGather / o_proj ======
            with (
                tc.tile_pool(name="at_mask", bufs=1) as mpool,
                tc.tile_pool(name="at_exp", bufs=3) as epool,
                tc.tile_pool(name="at_out", bufs=3) as opool,
                tc.tile_pool(name="at_r", bufs=2) as rpool,
                tc.tile_pool(name="op_w", bufs=1) as owpool,
                tc.tile_pool(name="op_ct", bufs=40) as ctpool,
                tc.tile_pool(name="op_stage", bufs=4) as ospool,
                tc.tile_pool(name="at_ps", bufs=2, space="PSUM") as aps,
                tc.tile_pool(name="op_ps", bufs=2, space="PSUM") as opspool,
            ):
                if mask_blocks:
                    mtile = mpool.tile([P, len(mask_blocks), SQ], F32,
                                       tag="mask")
                    for (t, b), i in mask_slot.items():
                        nc.sync.dma_start(
                            mtile[:, i, :],
                            maskT.ap()[t * P:(t + 1) * P,
                                       b * SQ:(b + 1) * SQ])
                wo_sb = owpool.tile([P, n_ht, dpc], MD, tag="wo")
                nc.sync.dma_start(wo_sb[:], woT_t[:])

                for b in range(n_sq):
                    ts_here = [t for t in range(n_st) if (t, b) in block_cls]
                    for h in range(hpc):
                        q_sl = qk_all[:, h, b * SQ:(b + 1) * SQ]
                        ps_o = aps.tile([P, SQ], F32, tag="out")
                        ps_row = aps.tile([P, SQ], F32, tag="row")
                        for i, t in enumerate(ts_here):
                            ps_s = aps.tile([P, SQ], F32, tag="scores")
                            nc.tensor.matmul(
                                ps_s[:],
                                qk_all[:, hpc + h, t * P:(t + 1) * P],
                                q_sl, start=True, stop=True)
                            if block_cls[(t, b)] == "mask":
                                nc.vector.tensor_add(
                                    ps_s[:], ps_s[:],
                                    mtile[:, mask_slot[(t, b)], :])
                            ex = epool.tile([P, SQ], MD, tag="exp")
                            nc.scalar.activation(
                                ex[:], ps_s[:],
                                mybir.ActivationFunctionType.Exp)
                            first, last = i == 0, i == len(ts_here) - 1
                            nc.tensor.matmul(
                                ps_o[:], v_sb[:, t, h * P:(h + 1) * P],
                                ex[:], start=first, stop=last)
                            # rowsum broadcast to all partitions via the
                            # all-ones stationary operand
                            nc.tensor.matmul(
                                ps_row[:], ones_sq[:], ex[:],
                                start=first, stop=last)
                        # evict ps_o to SBUF at once so the psum bank frees
                        # without waiting for the (slow) reciprocal
                        onum = rpool.tile([P, SQ], F32, tag="onum")
                        nc.vector.tensor_copy(onum[:], ps_o[:])
                        recip = rpool.tile([P, SQ], F32, tag="recip")
                        nc.vector.reciprocal(recip[:], ps_row[:])
                        ob = opool.tile([P, SQ], MD, tag="ob")
                        nc.vector.tensor_mul(ob[:], onum[:], recip[:])
                        nc.sync.dma_start(
                            gat_b[b][h // 2].ap()[(h % 2) * P:
                                                  (h % 2 + 1) * P, :], ob[:])
                        if h % 2 == 1:
                            nc.gpsimd.collective_compute(
                                "AllGather", mybir.AluOpType.bypass,
                                replica_groups=[list(range(N_CORES))],
                                ins=[gat_b[b][h // 2].ap().opt()],
                                outs=[ct_b[b][h // 2].ap().opt()])

                    # o_proj for this s_q block (overlaps later blocks);
                    # k-tile t of the half-gathers pairs with the
                    # host-permuted w_o row block t
                    cts = []
                    for pp in range(2):
                        ct_t = ct_b[b][pp].ap().rearrange(
                            "(t p) s -> p t s", p=P)
                        for t in range(n_ht // 2):
                            c_t = ctpool.tile([P, SQ], MD, tag="ct")
                            nc.sync.dma_start(c_t[:], ct_t[:, t, :])
                            cts.append(c_t)
                    for st in range(SQ // P):
                        ps = opspool.tile([P, dpc], F32, tag="op")
                        for t in range(n_ht):
                            nc.tensor.matmul(
                                ps[:],
                                cts[t][:, st * P:(st + 1) * P],
                                wo_sb[:, t, :],
                                start=(t == 0), stop=(t == n_ht - 1))
                        ob = ospool.tile([P, dpc], F32, tag="ostage")
                        nc.scalar.copy(ob[:], ps[:])
                        nc.sync.dma_start(
                            out_cols.ap()[b * SQ + st * P:
                                          b * SQ + (st + 1) * P, :], ob[:])

    nc.compile()
    return nc


def _classify_blocks(maskT_np, S):
    """Classify each [128, SQ] scoresT block of the (transposed) mask."""
    cls = {}
    for t in range(S // P):
        rows = maskT_np[t * P:(t + 1) * P]
        for b in range(S // SQ):
            blk = rows[:, b * SQ:(b + 1) * SQ]
            if np.all(blk <= -1e30):
                continue                      # fully masked: skip compute
            if np.all(blk == 0.0):
                cls[(t, b)] = "plain"
            else:
                cls[(t, b)] = "mask"
    return cls


def make_in_maps(hidden_states, attention_mask, w_pack, w_o):
    B, S, H = hidden_states.shape
    hpc = NUM_HEADS // N_CORES
    dpc = hpc * HEAD_DIM
    np_md = mybir.dt.np(_mm_dtype(MM_MODE))
    xT = np.ascontiguousarray(hidden_states[0].T).astype(np_md)
    maskT_np = np.ascontiguousarray(
        np.broadcast_to(attention_mask, (1, 1, S, S))[0, 0].T,
        dtype=np.float32)
    # w_o rows permuted to match the head-pair AllGather layout:
    # [pp][rank][head-in-pair] blocks of 128
    perm = np.concatenate(
        [np.arange(128 * (4 * r + 2 * pp + hh),
                   128 * (4 * r + 2 * pp + hh) + 128)
         for pp in (0, 1) for r in range(N_CORES) for hh in (0, 1)])
    in_maps = []
    for c in range(N_CORES):
        sl = slice(c * dpc, (c + 1) * dpc)
        wqk_c = np.concatenate(
            [w_pack[0 * H:1 * H][sl], w_pack[1 * H:2 * H][sl]], axis=0)
        woT_c = np.ascontiguousarray(w_o[sl].T)[perm]
        in_maps.append({
            "xT": xT,
            "wqkT": np.ascontiguousarray(wqk_c.T).astype(np_md),
            "wvT": np.ascontiguousarray(w_pack[2 * H:3 * H][sl].T
                                        ).astype(np_md),
            "maskT": maskT_np,
            "woT": np.ascontiguousarray(woT_c).astype(np_md),
        })
    return in_maps, maskT_np


def kernel(hidden_states, attention_mask, w_pack, w_o):
    B, S, H = hidden_states.shape
    assert B == 1 and H == NUM_HEADS * HEAD_DIM
    assert S % (2 * SQ) == 0

    in_maps, maskT_np = make_in_maps(hidden_states, attention_mask,
                                     w_pack, w_o)
    block_cls = _classify_blocks(maskT_np, S)

    key = (S, H, tuple(sorted(block_cls.items())), MM_MODE)
    if key not in _CACHE:
        _CACHE[key] = build(S, H, block_cls, MM_MODE)
    nc = _CACHE[key]

    res = run_bass_kernel_spmd(nc, in_maps, core_ids=list(range(N_CORES)))
    out = np.concatenate(
        [res.results[c]["out_cols"] for c in range(N_CORES)], axis=1)
    return out.reshape(1, S, H).astype(np.float32)


# revision 34
# speedup vs baseline: 1.1892x; 1.0732x over previous
"""Baichuan attention on 8 Trainium2 NeuronCores — tensor-parallel over heads.

Sharding: core c computes heads [4c, 4c+4): its slice of the fused QKV
projection, attention for those heads, then 1/8 of o_proj's output columns
after an AllGather of the per-core context slices (moves 4MB/rank instead of
a 32MB AllReduce of partial sums; mathematically identical to the module's
world_size logic).

Layout: scores are computed transposed (scoresT[k, q] blocks) so the PE
contraction dim always sits on SBUF partitions and every matmul streams a
512-wide moving operand. Matmul operands are fp16 (1 cyc/row on the PE) with
fp32 PSUM accumulation — measured end-to-end error vs the fp32 reference is
~6e-4 absmax-relative, on par with the f32r (tf32) path. The AllGather is
chunked over four s_q blocks so collective latency and o_proj overlap the
attention of later blocks, keeping the PE stream dense (HAM stays warm).
"""

import numpy as np

import concourse.bacc as bacc
import concourse.mybir as mybir
import concourse.tile as tile
from concourse.bass_utils import run_bass_kernel_spmd

F32 = mybir.dt.float32

N_CORES = 8
NUM_HEADS = 32
HEAD_DIM = 128
P = 128          # SBUF partitions / PE contraction tile
SQ = 512         # s_q block width (PSUM bank = 512 fp32)
MM_MODE = "f16"  # 'f16' | 'f32' (operand dtype for matmuls)

_CACHE: dict = {}


def _mm_dtype(mode):
    return {"f16": mybir.dt.float16, "f32": F32}[mode]


def build(S, H, block_cls, mode=MM_MODE):
    """Build the SPMD program. block_cls[(t, b)] = 'plain' | 'mask' for every
    computed scoresT block ([128 s_k] x [SQ s_q]); absent = fully masked, skip.
    """
    MD = _mm_dtype(mode)
    hpc = NUM_HEADS // N_CORES          # heads per core
    dpc = hpc * HEAD_DIM                # per-core slice of the hidden dim
    n_ht = H // P                       # contraction tiles for QKV/o_proj
    n_qk = 2 * dpc // P                 # q+k output tiles
    n_sq = S // SQ                      # s_q blocks
    n_st = S // P                       # s_k tiles
    scale = 1.0 / np.sqrt(np.float32(HEAD_DIM))
    s_half = S // 2
    sb_per_half = s_half // SQ

    nc = bacc.Bacc("TRN2", target_bir_lowering=False, debug=False,
                   num_devices=N_CORES)

    xT = nc.dram_tensor("xT", [H, S], MD, kind="ExternalInput")
    wqkT = nc.dram_tensor("wqkT", [H, 2 * dpc], MD, kind="ExternalInput")
    wvT = nc.dram_tensor("wvT", [H, dpc], MD, kind="ExternalInput")
    maskT = nc.dram_tensor("maskT", [S, S], F32, kind="ExternalInput")
    woT = nc.dram_tensor("woT", [H, dpc], MD, kind="ExternalInput")
    out_cols = nc.dram_tensor("out_cols", [S, dpc], F32, kind="ExternalOutput")

    # AllGather in head-pair chunks: gat[b][pp] holds local heads
    # {2pp, 2pp+1} for s_q block b; ct[b][pp] gathers those pairs from all
    # ranks. o_proj consumes them against host-permuted w_o rows.
    gat_b = [[nc.dram_tensor(f"gat_{b}_{pp}", [dpc // 2, SQ], MD)
              for pp in range(2)] for b in range(n_sq)]
    ct_b = [[nc.dram_tensor(f"ct_{b}_{pp}", [H // 2, SQ], MD,
                            addr_space="Shared") for pp in range(2)]
            for b in range(n_sq)]

    xT_t = xT.ap().rearrange("(t p) s -> p t s", p=P)
    wqkT_t = wqkT.ap().rearrange("(t p) o -> p t o", p=P)
    wvT_t = wvT.ap().rearrange("(t p) o -> p t o", p=P)
    woT_t = woT.ap().rearrange("(t p) j -> p t j", p=P)

    # sorted by (b, t) so block b=0's diagonal tiles arrive first
    mask_blocks = sorted({k for k, v in block_cls.items() if v == "mask"},
                         key=lambda k: (k[1], k[0]))
    mask_slot = {k: i for i, k in enumerate(mask_blocks)}

    with tile.TileContext(nc) as tc:
        with (
            tc.tile_pool(name="consts", bufs=1) as cpool,
            tc.tile_pool(name="span", bufs=1) as span,
        ):
            ones_f = cpool.tile([P, P], F32, tag="ones_f")
            nc.gpsimd.memset(ones_f[:], 1.0)
            ones_sq = cpool.tile([P, P], MD, tag="ones_sq")
            nc.scalar.copy(ones_sq[:], ones_f[:])

            # v ([s_k, d] natural, all heads) and q/k (transposed, all heads)
            # live in SBUF across phases 1-2; QKV evictions write them
            # directly (no DRAM bounce)
            v_sb = span.tile([P, n_st, dpc], MD, tag="v")
            qk_all = span.tile([P, n_qk, S], MD, tag="qk")

            # =============== phase 1: QKV projection ===============
            # q/k in transposed orientation -> DRAM scratch; v in natural
            # orientation (x stationary, Wv moving) -> resident v_sb.
            with (
                tc.tile_pool(name="qkv_x", bufs=1) as xpool,
                tc.tile_pool(name="qkv_w", bufs=3) as wpool,
                tc.tile_pool(name="qkv_wv", bufs=1) as wvpool,
                tc.tile_pool(name="qkv_ps", bufs=4, space="PSUM") as pspool,
            ):
                wv_sb = wvpool.tile([P, n_ht, dpc], MD, tag="wv")
                for half in range(2):
                    # first q/k weight tile ahead of the bulk x transfer so
                    # the PE starts within a few us; wv's 4MB comes after the
                    # second w tile. x arrives as two SQ-wide quarters so the
                    # next half's load overlaps this half's tail.
                    w_tiles = {}
                    w_tiles[0] = wpool.tile([P, n_ht, P], MD, tag="w",
                                            name="w_tile")
                    nc.sync.dma_start(
                        w_tiles[0][:], wqkT_t[:, :, 0 * P:1 * P])
                    xq = []
                    for sb in range(sb_per_half):
                        x_tile = xpool.tile([P, n_ht, SQ], MD, tag="x",
                                            bufs=3, name="x_tile")
                        lo = half * s_half + sb * SQ
                        for t in range(n_ht):
                            nc.sync.dma_start(
                                x_tile[:, t, :], xT_t[:, t, lo:lo + SQ])
                        xq.append(x_tile)
                    for ot in range(n_qk):
                        if ot not in w_tiles:
                            w_tiles[ot] = wpool.tile([P, n_ht, P], MD,
                                                     tag="w", name="w_tile")
                            nc.sync.dma_start(
                                w_tiles[ot][:],
                                wqkT_t[:, :, ot * P:(ot + 1) * P])
                        w_tile = w_tiles[ot]
                        if half == 0 and ot == 1:
                            nc.sync.dma_start(wv_sb[:], wvT_t[:])
                        for sb in range(sb_per_half):
                            ps = pspool.tile([P, SQ], F32, tag="qkv")
                            for t in range(n_ht):
                                nc.tensor.matmul(
                                    ps[:],
                                    w_tile[:, t, :],
                                    xq[sb][:, t, :],
                                    start=(t == 0), stop=(t == n_ht - 1))
                            # fold the softmax scale into q at eviction;
                            # write straight into the resident qk tile
                            mul = scale if ot < dpc // P else 1.0
                            lo = half * s_half + sb * SQ
                            nc.scalar.mul(qk_all[:, ot, lo:lo + SQ],
                                          ps[:], mul)
                    # v: psum [s=128, dpc] accumulated over h-tiles
                    for sti in range(s_half // P):
                        st_g = half * (s_half // P) + sti
                        sb, off = (sti * P) // SQ, (sti * P) % SQ
                        ps_v = pspool.tile([P, dpc], F32, tag="qkv")
                        for t in range(n_ht):
                            nc.tensor.matmul(
                                ps_v[:],
                                xq[sb][:, t, off:off + P],
                                wv_sb[:, t, :],
                                start=(t == 0), stop=(t == n_ht - 1))
                        nc.vector.tensor_copy(v_sb[:, st_g, :], ps_v[:])

            # ====== phases 2-4: attention / chunked AllGather / o_proj ======
            with (
                tc.tile_pool(name="at_mask", bufs=1) as mpool,
                tc.tile_pool(name="at_exp", bufs=3) as epool,
                tc.tile_pool(name="at_out", bufs=3) as opool,
                tc.tile_pool(name="at_r", bufs=2) as rpool,
                tc.tile_pool(name="op_w", bufs=1) as owpool,
                tc.tile_pool(name="op_ct", bufs=40) as ctpool,
                tc.tile_pool(name="op_stage", bufs=4) as ospool,
                tc.tile_pool(name="at_ps", bufs=2, space="PSUM") as aps,
                tc.tile_pool(name="op_ps", bufs=2, space="PSUM") as opspool,
            ):
                if mask_blocks:
                    mtile = mpool.tile([P, len(mask_blocks), SQ], F32,
                                       tag="mask")
                    for (t, b), i in mask_slot.items():
                        nc.sync.dma_start(
                            mtile[:, i, :],
                            maskT.ap()[t * P:(t + 1) * P,
                                       b * SQ:(b + 1) * SQ])
                wo_sb = owpool.tile([P, n_ht, dpc], MD, tag="wo")

                def emit_oproj(b, cts):
                    for st in range(SQ // P):
                        ps = opspool.tile([P, dpc], F32, tag="op",
                                          name="op_ps")
                        for t in range(n_ht):
                            nc.tensor.matmul(
                                ps[:],
                                cts[t][:, st * P:(st + 1) * P],
                                wo_sb[:, t, :],
                                start=(t == 0), stop=(t == n_ht - 1))
                        ob = ospool.tile([P, dpc], F32, tag="ostage",
                                         name="ostage")
                        nc.scalar.copy(ob[:], ps[:])
                        nc.sync.dma_start(
                            out_cols.ap()[b * SQ + st * P:
                                          b * SQ + (st + 1) * P, :], ob[:])

                pending = []
                for b in range(n_sq):
                    ts_here = [t for t in range(n_st) if (t, b) in block_cls]
                    for h in range(hpc):
                        q_sl = qk_all[:, h, b * SQ:(b + 1) * SQ]
                        ps_o = aps.tile([P, SQ], F32, tag="out")
                        ps_row = aps.tile([P, SQ], F32, tag="row")
                        for i, t in enumerate(ts_here):
                            ps_s = aps.tile([P, SQ], F32, tag="scores")
                            nc.tensor.matmul(
                                ps_s[:],
                                qk_all[:, hpc + h, t * P:(t + 1) * P],
                                q_sl, start=True, stop=True)
                            if block_cls[(t, b)] == "mask":
                                nc.vector.tensor_add(
                                    ps_s[:], ps_s[:],
                                    mtile[:, mask_slot[(t, b)], :])
                            ex = epool.tile([P, SQ], MD, tag="exp")
                            nc.scalar.activation(
                                ex[:], ps_s[:],
                                mybir.ActivationFunctionType.Exp)
                            first, last = i == 0, i == len(ts_here) - 1
                            nc.tensor.matmul(
                                ps_o[:], v_sb[:, t, h * P:(h + 1) * P],
                                ex[:], start=first, stop=last)
                            # rowsum broadcast to all partitions via the
                            # all-ones stationary operand
                            nc.tensor.matmul(
                                ps_row[:], ones_sq[:], ex[:],
                                start=first, stop=last)
                        # evict ps_o to SBUF at once so the psum bank frees
                        # without waiting for the (slow) reciprocal
                        onum = rpool.tile([P, SQ], F32, tag="onum")
                        nc.vector.tensor_copy(onum[:], ps_o[:])
                        recip = rpool.tile([P, SQ], F32, tag="recip")
                        nc.vector.reciprocal(recip[:], ps_row[:])
                        ob = opool.tile([P, SQ], MD, tag="ob")
                        nc.vector.tensor_mul(ob[:], onum[:], recip[:])
                        nc.sync.dma_start(
                            gat_b[b][h // 2].ap()[(h % 2) * P:
                                                  (h % 2 + 1) * P, :], ob[:])
                        if h % 2 == 1:
                            nc.gpsimd.collective_compute(
                                "AllGather", mybir.AluOpType.bypass,
                                replica_groups=[list(range(N_CORES))],
                                ins=[gat_b[b][h // 2].ap().opt()],
                                outs=[ct_b[b][h // 2].ap().opt()])

                    if b == 0:
                        # wo arrives during the first AllGather, off the
                        # startup critical path
                        nc.sync.dma_start(wo_sb[:], woT_t[:])

                    # prefetch this block's gathered context tiles; k-tile t
                    # of the half-gathers pairs with the host-permuted w_o
                    # row block t
                    cts = []
                    for pp in range(2):
                        ct_t = ct_b[b][pp].ap().rearrange(
                            "(t p) s -> p t s", p=P)
                        for t in range(n_ht // 2):
                            c_t = ctpool.tile([P, SQ], MD, tag="ct")
                            nc.sync.dma_start(c_t[:], ct_t[:, t, :])
                            cts.append(c_t)
                    # o_proj for block b is emitted after attention b+1 so
                    # the PE prefers attention work and o_proj acts as filler
                    pending.append((b, cts))
                    if len(pending) == 2:
                        emit_oproj(*pending.pop(0))
                for bb, ccts in pending:
                    emit_oproj(bb, ccts)

    nc.compile()
    return nc


def _classify_blocks(maskT_np, S):
    """Classify each [128, SQ] scoresT block of the (transposed) mask."""
    cls = {}
    for t in range(S // P):
        rows = maskT_np[t * P:(t + 1) * P]
        for b in range(S // SQ):
            blk = rows[:, b * SQ:(b + 1) * SQ]
            if np.all(blk <= -1e30):
                continue                      # fully masked: skip compute
            if np.all(blk == 0.0):
                cls[(t, b)] = "plain"
            else:
                cls[(t, b)] = "mask"
    return cls


def make_in_maps(hidden_states, attention_mask, w_pack, w_o):
    B, S, H = hidden_states.shape
    hpc = NUM_HEADS // N_CORES
    dpc = hpc * HEAD_DIM
    np_md = mybir.dt.np(_mm_dtype(MM_MODE))
    xT = np.ascontiguousarray(hidden_states[0].T).astype(np_md)
    maskT_np = np.ascontiguousarray(
        np.broadcast_to(attention_mask, (1, 1, S, S))[0, 0].T,
        dtype=np.float32)
    # w_o rows permuted to match the head-pair AllGather layout:
    # [pp][rank][head-in-pair] blocks of 128
    perm = np.concatenate(
        [np.arange(128 * (4 * r + 2 * pp + hh),
                   128 * (4 * r + 2 * pp + hh) + 128)
         for pp in (0, 1) for r in range(N_CORES) for hh in (0, 1)])
    in_maps = []
    for c in range(N_CORES):
        sl = slice(c * dpc, (c + 1) * dpc)
        wqk_c = np.concatenate(
            [w_pack[0 * H:1 * H][sl], w_pack[1 * H:2 * H][sl]], axis=0)
        woT_c = np.ascontiguousarray(w_o[sl].T)[perm]
        in_maps.append({
            "xT": xT,
            "wqkT": np.ascontiguousarray(wqk_c.T).astype(np_md),
            "wvT": np.ascontiguousarray(w_pack[2 * H:3 * H][sl].T
                                        ).astype(np_md),
            "maskT": maskT_np,
            "woT": np.ascontiguousarray(woT_c).astype(np_md),
        })
    return in_maps, maskT_np


def kernel(hidden_states, attention_mask, w_pack, w_o):
    B, S, H = hidden_states.shape
    assert B == 1 and H == NUM_HEADS * HEAD_DIM
    assert S % (2 * SQ) == 0

    in_maps, maskT_np = make_in_maps(hidden_states, attention_mask,
                                     w_pack, w_o)
    block_cls = _classify_blocks(maskT_np, S)

    key = (S, H, tuple(sorted(block_cls.items())), MM_MODE)
    if key not in _CACHE:
        _CACHE[key] = build(S, H, block_cls, MM_MODE)
    nc = _CACHE[key]

    res = run_bass_kernel_spmd(nc, in_maps, core_ids=list(range(N_CORES)))
    out = np.concatenate(
        [res.results[c]["out_cols"] for c in range(N_CORES)], axis=1)
    return out.reshape(1, S, H).astype(np.float32)
